# revision 3
# baseline (speedup 1.0000x reference)
"""GraphSAGE 2-layer mean-aggregation kernel for 8 Trainium2 NeuronCores.

Problem (full shapes):
    features [2_000_000, 128] f32, samples0 [1024], samples1 [1024, 25],
    samples2 [1024, 25, 10] -> out [1024, 256] f32.

Strategy (v3 — fp16 feature-major staging, contiguous DVE trees):
  * Data-parallel over the batch: core c handles batches [128c, 128c+128).
  * Per the sharding_hint's all-to-all gather, each core is staged exactly
    the rows its samples reference — TRANSPOSED (feature-major) in fp16.
    h2 chunks are staged (s2, s1, batch) so every s2-tree add is a fully
    contiguous half-buffer + half-buffer (DVE 2x 16-bit mode).
  * Device pipeline per h2 chunk (<=4 s1-slices):
      DMA chunk (SP/ACT queues alternate, issue 3 chunks ahead)
      -> DVE s2 sum tree -> PE fp16 projections (ws0 / wn0, scales 1/S2,
      1/S1 folded on host) -> ACT relu -> GpSimd accumulates
      sum_s1 relu(n1) into macc.
  * Tail: mean_s1(h1) tree (GpSimd, mid-stream), n0 projections, layer-1
    matmuls, relu, out [128, 256] f32 DMA.  Host un-transposes the output.

Self-contained: hardcodes all shapes; only needs numpy + the concourse
(Bass) stack that is on the container's default python path.
"""

import sys

for _p in ("/opt/trn_rl_repo",):
    if _p not in sys.path:
        sys.path.append(_p)

import numpy as np

import concourse.bass as bass
import concourse.mybir as mybir
import concourse.tile as tile
from concourse import bacc
from concourse.bass_utils import run_bass_kernel_spmd

F32 = mybir.dt.float32
F16 = mybir.dt.float16
RELU = mybir.ActivationFunctionType.Relu

N_CORES = 8
B = 1024
BL = B // N_CORES          # 128 batches per core
S1, S2 = 25, 10
D = 128                    # feature dim = OUT0 = OUT1 = 128
CHUNK_SLS = (3, 4, 4, 4, 4, 4, 2)
CHUNK_OFF = (0, 3, 7, 11, 15, 19, 23)
H2_CHUNKS = len(CHUNK_SLS)
N_H1 = BL * S1                        # 3200 cols
N_H2 = BL * S1 * S2                   # 32000 cols
W_NAMES = ("ws0", "wn0s2", "wn0s1", "ws1a", "ws1b", "wn1a", "wn1b")


def build_bass() -> bass.Bass:
    nc = bacc.Bacc()

    # feature-major fp16 staged tables: row = feature.
    # h2 cols per chunk are (s2, s1_local, batch); h1 cols are (s1, batch).
    h2_d = nc.dram_tensor("h2", [D, N_H2], F16, kind="ExternalInput")
    h1_d = nc.dram_tensor("h1", [D, N_H1], F16, kind="ExternalInput")
    # [h0T (128 cols) | 7 weight blocks of 128 cols]
    h0w_d = nc.dram_tensor("h0w", [D, (1 + len(W_NAMES)) * D], F16,
                           kind="ExternalInput")
    out_d = nc.dram_tensor("out", [D, 2 * BL], F32, kind="ExternalOutput")

    with tile.TileContext(nc) as tc:
        with (
            tc.tile_pool(name="const", bufs=1) as cpool,
            tc.tile_pool(name="h2", bufs=3) as h2pool,
            tc.tile_pool(name="sb", bufs=2) as sbpool,
            tc.tile_pool(name="ps", bufs=2, space="PSUM") as pspool,
        ):
            def dma_h2(c):
                nsl = CHUNK_SLS[c]
                t = h2pool.tile([D, S2, nsl, BL], F16, tag="h2c")
                c0 = CHUNK_OFF[c] * S2 * BL
                eng = nc.scalar if c % 2 == 0 else nc.sync
                eng.dma_start(
                    t[:],
                    h2_d[:, c0:c0 + nsl * S2 * BL].rearrange(
                        "p (t s b) -> p t s b", t=S2, s=nsl),
                )
                return t

            # weights + h0 first on SP (needed by every matmul), then h1;
            # first three h2 chunks in flight immediately.
            h0w = cpool.tile([D, (1 + len(W_NAMES)) * D], F16, tag="h0w")
            nc.sync.dma_start(h0w[:], h0w_d[:])
            h0 = h0w[:, 0:D]
            w = {name: h0w[:, (1 + i) * D:(2 + i) * D]
                 for i, name in enumerate(W_NAMES)}
            pre = [dma_h2(0)]
            h1t = cpool.tile([D, S1, BL], F16, tag="h1")
            nc.sync.dma_start(
                h1t[:], h1_d[:].rearrange("p (s b) -> p s b", s=S1))
            pre += [dma_h2(1), dma_h2(2)]

            # sum_s1 relu(n1T) accumulator (GpSimd); cols = (half, batch);
            # 1/S1 folded into wn1 on the host.
            macc = cpool.tile([D, 2 * BL], F16, tag="macc")

            for c in range(H2_CHUNKS):
                nsl = CHUNK_SLS[c]
                v = pre[c] if c < 3 else None
                if c + 3 < H2_CHUNKS:
                    pre.append(dma_h2(c + 3))
                if v is None:
                    v = pre[c]
                # s2 sum tree on DVE — every add is contiguous halves
                a5 = sbpool.tile([D, 5, nsl, BL], F16, tag="a5")
                nc.vector.tensor_add(a5[:], v[:, 0:5, :, :], v[:, 5:10, :, :])
                a2 = sbpool.tile([D, 2, nsl, BL], F16, tag="a2")
                nc.vector.tensor_add(a2[:], a5[:, 0:2, :, :], a5[:, 2:4, :, :])
                m2 = sbpool.tile([D, nsl, BL], F16, tag="m2")
                nc.vector.tensor_add(m2[:], a2[:, 0, :, :], a2[:, 1, :, :])
                nc.vector.tensor_add(m2[:], m2[:], a5[:, 4, :, :])

                # projections: self = ws0^T h1T, neigh = (wn0/S2)^T m2
                s0 = CHUNK_OFF[c]
                ps_s = pspool.tile([D, 4 * BL], F32, tag="ps_s")
                nc.tensor.matmul(
                    ps_s[:, 0:nsl * BL], lhsT=w["ws0"],
                    rhs=h1t[:, s0:s0 + nsl, :], start=True, stop=True)
                ps_n = pspool.tile([D, 4 * BL], F32, tag="ps_n")
                nc.tensor.matmul(
                    ps_n[:, 0:nsl * BL], lhsT=w["wn0s2"],
                    rhs=m2[:], start=True, stop=True)

                # relu into rn; cols = (s, half, batch)
                rn = sbpool.tile([D, nsl, 2, BL], F16, tag="rn")
                nc.scalar.activation(
                    rn[:, :, 0, :],
                    ps_s[:, 0:nsl * BL].rearrange("p (s b) -> p s b", s=nsl),
                    RELU)
                nc.scalar.activation(
                    rn[:, :, 1, :],
                    ps_n[:, 0:nsl * BL].rearrange("p (s b) -> p s b", s=nsl),
                    RELU)

                # macc += sum_s rn[:, s, :, :] on GpSimd (idle engine)
                mview = macc[:].rearrange("p (a b) -> p a b", a=2)
                x = sbpool.tile([D, 2, 2, BL], F16, tag="xpair")
                if nsl == 4:
                    nc.gpsimd.tensor_add(
                        x[:], rn[:, 0:2, :, :], rn[:, 2:4, :, :])
                    nc.gpsimd.tensor_add(x[:, 0, :, :], x[:, 0, :, :],
                                         x[:, 1, :, :])
                    nc.gpsimd.tensor_add(mview, mview, x[:, 0, :, :])
                elif nsl == 3:
                    nc.gpsimd.tensor_add(
                        x[:, 0, :, :], rn[:, 0, :, :], rn[:, 1, :, :])
                    nc.gpsimd.tensor_add(x[:, 0, :, :], x[:, 0, :, :],
                                         rn[:, 2, :, :])
                    # chunk 0 initializes macc
                    nc.gpsimd.tensor_copy(out=macc[:],
                                          in_=x[:, 0, :, :].rearrange(
                                              "p a b -> p (a b)"))
                else:  # nsl == 2
                    nc.gpsimd.tensor_add(
                        x[:, 0, :, :], rn[:, 0, :, :], rn[:, 1, :, :])
                    nc.gpsimd.tensor_add(mview, mview, x[:, 0, :, :])

                if c == 1:
                    # mean_s1 h1 for n0 (1/S1 folded into wn0s1): GpSimd
                    # tree over the 25 h1 slices, mid-stream (h1 landed).
                    t12 = cpool.tile([D, 12, BL], F16, tag="t12")
                    nc.gpsimd.tensor_add(t12[:], h1t[:, 0:12, :],
                                         h1t[:, 12:24, :])
                    t6 = cpool.tile([D, 6, BL], F16, tag="t6")
                    nc.gpsimd.tensor_add(t6[:], t12[:, 0:6, :],
                                         t12[:, 6:12, :])
                    t3 = cpool.tile([D, 3, BL], F16, tag="t3")
                    nc.gpsimd.tensor_add(t3[:], t6[:, 0:3, :], t6[:, 3:6, :])
                    mh1 = cpool.tile([D, BL], F16, tag="mh1")
                    nc.gpsimd.tensor_add(mh1[:], t3[:, 0, :], t3[:, 1, :])
                    nc.gpsimd.tensor_add(mh1[:], mh1[:], t3[:, 2, :])
                    nc.gpsimd.tensor_add(mh1[:], mh1[:], h1t[:, 24, :])

            # ---- tail: n0 and layer 1 ----
            ps_0 = pspool.tile([D, 2 * BL], F32, tag="ps_t")
            nc.tensor.matmul(ps_0[:, 0:BL], lhsT=w["ws0"], rhs=h0,
                             start=True, stop=True)
            nc.tensor.matmul(ps_0[:, BL:2 * BL], lhsT=w["wn0s1"], rhs=mh1[:],
                             start=True, stop=True)
            n0 = cpool.tile([D, 2 * BL], F16, tag="n0")
            nc.scalar.activation(n0[:], ps_0[:], RELU)

            ps_1 = pspool.tile([D, 2 * BL], F32, tag="ps_t")
            nc.tensor.matmul(ps_1[:, 0:BL], lhsT=w["ws1a"], rhs=n0[:, 0:BL],
                             start=True, stop=False)
            nc.tensor.matmul(ps_1[:, 0:BL], lhsT=w["ws1b"],
                             rhs=n0[:, BL:2 * BL], start=False, stop=True)
            nc.tensor.matmul(ps_1[:, BL:2 * BL], lhsT=w["wn1a"],
                             rhs=macc[:, 0:BL], start=True, stop=False)
            nc.tensor.matmul(ps_1[:, BL:2 * BL], lhsT=w["wn1b"],
                             rhs=macc[:, BL:2 * BL], start=False, stop=True)
            ofin = cpool.tile([D, 2 * BL], F32, tag="ofin")
            nc.scalar.activation(ofin[:], ps_1[:], RELU)
            nc.sync.dma_start(out_d[:], ofin[:])

    nc.compile()
    return nc


def make_in_maps(inputs: dict) -> list[dict]:
    feat16 = np.asarray(inputs["features"]).astype(np.float16)
    s0 = np.asarray(inputs["samples0"]).astype(np.int64).reshape(B)
    s1 = np.asarray(inputs["samples1"]).astype(np.int64).reshape(B, S1)
    s2 = np.asarray(inputs["samples2"]).astype(np.int64).reshape(B, S1, S2)
    ws0 = np.asarray(inputs["w_self0"], dtype=np.float32)
    wn0 = np.asarray(inputs["w_neigh0"], dtype=np.float32)
    ws1 = np.asarray(inputs["w_self1"], dtype=np.float32)
    wn1 = np.asarray(inputs["w_neigh1"], dtype=np.float32)

    # order must match W_NAMES; scales folded: wn0s2 = wn0/S2 (n1 neigh),
    # wn0s1 = wn0/S1 (n0 neigh, h1 tree is a plain sum), wn1*/S1.
    w_cat = np.concatenate([
        ws0, wn0 / S2, wn0 / S1, ws1[:D], ws1[D:], wn1[:D] / S1,
        wn1[D:] / S1,
    ], axis=1).astype(np.float16)   # [128, 7*128]

    in_maps = []
    for c in range(N_CORES):
        b0 = c * BL
        # h2T: per chunk, cols = (s2, s1_local, batch)
        s2c = s2[b0:b0 + BL]                         # [BL, S1, S2]
        idx_parts = []
        for cc in range(H2_CHUNKS):
            o, nsl = CHUNK_OFF[cc], CHUNK_SLS[cc]
            blk = s2c[:, o:o + nsl, :]               # [BL, nsl, S2]
            idx_parts.append(blk.transpose(2, 1, 0).reshape(-1))
        ids2 = np.concatenate(idx_parts)
        h2T = np.ascontiguousarray(feat16[ids2].T)   # [128, 32000]
        ids1 = s1[b0:b0 + BL].T.reshape(-1)          # (s1, b) flat
        h1T = np.ascontiguousarray(feat16[ids1].T)   # [128, 3200]
        h0T = feat16[s0[b0:b0 + BL]].T               # [128, 128]
        h0w = np.ascontiguousarray(
            np.concatenate([h0T, w_cat], axis=1))    # [128, 1024]
        in_maps.append(dict(h2=h2T, h1=h1T, h0w=h0w))
    return in_maps


_NC_CACHE = None


def _get_nc() -> bass.Bass:
    global _NC_CACHE
    if _NC_CACHE is None:
        _NC_CACHE = build_bass()
    return _NC_CACHE


def run(inputs: dict, trace: bool = False):
    """Returns (full_output [1024, 256] f32, BassKernelResults)."""
    in_maps = make_in_maps(inputs)
    res = run_bass_kernel_spmd(
        _get_nc(), in_maps, core_ids=list(range(N_CORES)), trace=trace
    )
    # device out r[j, half*128+b] -> out[b, half*128+j]
    outs = []
    for r in res.results:
        r2 = np.asarray(r["out"]).reshape(D, 2, BL)
        outs.append(r2.transpose(2, 1, 0).reshape(BL, 2 * D))
    return np.concatenate(outs, axis=0), res


def kernel(**inputs) -> np.ndarray:
    out, _ = run(inputs)
    return out


# revision 7
# speedup vs baseline: 1.3485x; 1.3485x over previous
"""GraphSAGE 2-layer mean-aggregation kernel for 8 Trainium2 NeuronCores.

Problem (full shapes):
    features [2_000_000, 128] f32, samples0 [1024], samples1 [1024, 25],
    samples2 [1024, 25, 10] -> out [1024, 256] f32.

Strategy (v5 — fp16 feature-major staging, engine-balanced pipeline):
  * Data-parallel over the batch: core c handles batches [128c, 128c+128).
  * Per the sharding_hint's all-to-all gather, each core is staged exactly
    the rows its samples reference — TRANSPOSED (feature-major) in fp16,
    h2 chunks in (s2, s1, batch) order so every reduction is contiguous.
  * Engine split per h2 chunk (<=4 s1-slices):
      DMA chunk (SP/ACT queues alternate, issue 3 ahead)
      -> DVE s2 tree levels 1-2 (fp16 2x mode)
      -> GpSimd tree levels 3-4 (m2)
      -> PE fp16 projections (ws0 / wn0; 1/S2, 1/S1 folded on host)
      -> ACT relu -> PE identity-matmul accumulates sum_s1 relu(n1) into a
         dedicated PSUM bank (skewed one chunk so PE never waits on ACT).
  * mean_s1(h1) tree (DVE) and the n0 projections run mid-stream; the tail
    is only the layer-1 matmuls + relu + out DMA.  Host un-transposes the
    tiny [128, 256] output.

Self-contained: hardcodes all shapes; only needs numpy + the concourse
(Bass) stack that is on the container's default python path.
"""

import sys

for _p in ("/opt/trn_rl_repo",):
    if _p not in sys.path:
        sys.path.append(_p)

import numpy as np

import concourse.bass as bass
import concourse.mybir as mybir
import concourse.tile as tile
from concourse import bacc
from concourse.bass_utils import run_bass_kernel_spmd

F32 = mybir.dt.float32
F16 = mybir.dt.float16
RELU = mybir.ActivationFunctionType.Relu
COPY = mybir.ActivationFunctionType.Copy

N_CORES = 8
B = 1024
BL = B // N_CORES          # 128 batches per core
S1, S2 = 25, 10
D = 128                    # feature dim = OUT0 = OUT1 = 128
CHUNK_SLS = (3, 4, 4, 4, 4, 4, 1, 1)
CHUNK_OFF = (0, 3, 7, 11, 15, 19, 23, 24)
H2_CHUNKS = len(CHUNK_SLS)
N_H1 = BL * S1                        # 3200 cols
N_H2 = BL * S1 * S2                   # 32000 cols
W_NAMES = ("ws0", "wn0s2", "wn0s1", "ws1a", "ws1b", "wn1a", "wn1b", "ident")


def build_bass() -> bass.Bass:
    nc = bacc.Bacc()

    # feature-major fp16 staged tables: row = feature.
    # h2 cols per chunk are (s2, s1_local, batch); h1 cols are (s1, batch).
    h2_d = nc.dram_tensor("h2", [D, N_H2], F16, kind="ExternalInput")
    h1_d = nc.dram_tensor("h1", [D, N_H1], F16, kind="ExternalInput")
    # [h0T (128 cols) | 8 weight blocks of 128 cols]
    h0w_d = nc.dram_tensor("h0w", [D, (1 + len(W_NAMES)) * D], F16,
                           kind="ExternalInput")
    out_d = nc.dram_tensor("out", [D, 2 * BL], F32, kind="ExternalOutput")

    with tile.TileContext(nc) as tc:
        with (
            tc.tile_pool(name="const", bufs=1) as cpool,
            tc.tile_pool(name="h2", bufs=3) as h2pool,
            tc.tile_pool(name="sb", bufs=2) as sbpool,
            tc.tile_pool(name="ps", bufs=2, space="PSUM") as pspool,
            tc.tile_pool(name="psa", bufs=1, space="PSUM") as psapool,
        ):
            def dma_h2(c):
                nsl = CHUNK_SLS[c]
                t = h2pool.tile([D, S2, nsl, BL], F16, tag="h2c")
                c0 = CHUNK_OFF[c] * S2 * BL
                eng = nc.sync if c % 2 == 0 else nc.scalar
                eng.dma_start(
                    t[:],
                    h2_d[:, c0:c0 + nsl * S2 * BL].rearrange(
                        "p (t s b) -> p t s b", t=S2, s=nsl),
                )
                return t

            # stream starts immediately: chunk 0/1 are the first
            # instructions on their queues; weights + h1 right behind.
            pre = [dma_h2(0), dma_h2(1)]
            h0w = cpool.tile([D, (1 + len(W_NAMES)) * D], F16, tag="h0w")
            nc.sync.dma_start(h0w[:], h0w_d[:])
            h1t = cpool.tile([D, S1, BL], F16, tag="h1")
            nc.sync.dma_start(
                h1t[:], h1_d[:].rearrange("p (s b) -> p s b", s=S1))
            h0 = h0w[:, 0:D]
            w = {name: h0w[:, (1 + i) * D:(2 + i) * D]
                 for i, name in enumerate(W_NAMES)}
            pre += [dma_h2(2)]

            # PSUM accumulator for sum_s1 relu(n1T); cols = (half, batch);
            # 1/S1 folded into wn1 on the host.
            macc_ps = psapool.tile([D, 2, BL], F32, tag="macc_ps")
            n_mm = 0

            def macc_mms(rn_p, nsl_p):
                nonlocal n_mm
                for sl in range(nsl_p):
                    nc.tensor.matmul(
                        macc_ps[:], lhsT=w["ident"], rhs=rn_p[:, sl, :, :],
                        start=(n_mm == 0), stop=(n_mm == S1 - 1),
                        skip_group_check=True)
                    n_mm += 1

            rn_prev = None
            for c in range(H2_CHUNKS):
                nsl = CHUNK_SLS[c]
                v = pre[c]
                if c + 3 < H2_CHUNKS:
                    pre.append(dma_h2(c + 3))
                # skewed: accumulate previous chunk's relu(n1) on PE first
                if rn_prev is not None:
                    macc_mms(rn_prev, CHUNK_SLS[c - 1])
                # s2 tree on DVE (contiguous halves, fp16 2x mode)
                a5 = sbpool.tile([D, 5, nsl, BL], F16, tag="a5")
                nc.vector.tensor_add(a5[:], v[:, 0:5, :, :], v[:, 5:10, :, :])
                a2 = sbpool.tile([D, 2, nsl, BL], F16, tag="a2")
                nc.vector.tensor_add(a2[:], a5[:, 0:2, :, :], a5[:, 2:4, :, :])
                m2 = sbpool.tile([D, nsl, BL], F16, tag="m2")
                nc.vector.tensor_add(m2[:], a2[:, 0, :, :], a2[:, 1, :, :])
                nc.vector.tensor_add(m2[:], m2[:], a5[:, 4, :, :])
                # mean_s1(h1) tree: one DVE op per chunk, c = 1..5
                if c == 1:
                    t12 = cpool.tile([D, 12, BL], F16, tag="t12")
                    nc.vector.tensor_add(t12[:], h1t[:, 0:12, :],
                                         h1t[:, 12:24, :])
                elif c == 2:
                    t6 = cpool.tile([D, 6, BL], F16, tag="t6")
                    nc.vector.tensor_add(t6[:], t12[:, 0:6, :],
                                         t12[:, 6:12, :])
                elif c == 3:
                    t3 = cpool.tile([D, 3, BL], F16, tag="t3")
                    nc.vector.tensor_add(t3[:], t6[:, 0:3, :], t6[:, 3:6, :])
                elif c == 4:
                    mh1 = cpool.tile([D, BL], F16, tag="mh1")
                    nc.vector.tensor_add(mh1[:], t3[:, 0, :], t3[:, 1, :])
                elif c == 5:
                    nc.vector.tensor_add(mh1[:], mh1[:], t3[:, 2, :])
                    nc.vector.tensor_add(mh1[:], mh1[:], h1t[:, 24, :])

                # projections: self = ws0^T h1T, neigh = (wn0/S2)^T m2
                s0 = CHUNK_OFF[c]
                ps_s = pspool.tile([D, 4 * BL], F32, tag="ps_s")
                nc.tensor.matmul(
                    ps_s[:, 0:nsl * BL], lhsT=w["ws0"],
                    rhs=h1t[:, s0:s0 + nsl, :], start=True, stop=True)
                ps_n = pspool.tile([D, 4 * BL], F32, tag="ps_n")
                nc.tensor.matmul(
                    ps_n[:, 0:nsl * BL], lhsT=w["wn0s2"],
                    rhs=m2[:], start=True, stop=True)

                # relu into rn; cols = (s, half, batch)
                rn = sbpool.tile([D, nsl, 2, BL], F16, tag="rn")
                nc.scalar.activation(
                    rn[:, :, 0, :],
                    ps_s[:, 0:nsl * BL].rearrange("p (s b) -> p s b", s=nsl),
                    RELU)
                nc.scalar.activation(
                    rn[:, :, 1, :],
                    ps_n[:, 0:nsl * BL].rearrange("p (s b) -> p s b", s=nsl),
                    RELU)
                rn_prev = rn

                if c == 5:
                    # n0 projections mid-stream (mh1 just became ready)
                    ps_0 = pspool.tile([D, 2 * BL], F32, tag="ps_t")
                    nc.tensor.matmul(ps_0[:, 0:BL], lhsT=w["ws0"], rhs=h0,
                                     start=True, stop=True)
                    nc.tensor.matmul(ps_0[:, BL:2 * BL], lhsT=w["wn0s1"],
                                     rhs=mh1[:], start=True, stop=True)
                    n0 = cpool.tile([D, 2 * BL], F16, tag="n0")
                    nc.scalar.activation(n0[:], ps_0[:], RELU)

            # ---- tail ----
            macc_mms(rn_prev, CHUNK_SLS[-1])
            macc = cpool.tile([D, 2 * BL], F16, tag="macc")
            nc.scalar.activation(
                macc[:], macc_ps[:].rearrange("p a b -> p (a b)"), COPY)

            ps_1 = pspool.tile([D, 2 * BL], F32, tag="ps_t")
            nc.tensor.matmul(ps_1[:, 0:BL], lhsT=w["ws1a"], rhs=n0[:, 0:BL],
                             start=True, stop=False)
            nc.tensor.matmul(ps_1[:, 0:BL], lhsT=w["ws1b"],
                             rhs=n0[:, BL:2 * BL], start=False, stop=True)
            nc.tensor.matmul(ps_1[:, BL:2 * BL], lhsT=w["wn1a"],
                             rhs=macc[:, 0:BL], start=True, stop=False)
            nc.tensor.matmul(ps_1[:, BL:2 * BL], lhsT=w["wn1b"],
                             rhs=macc[:, BL:2 * BL], start=False, stop=True)
            ofin = cpool.tile([D, 2 * BL], F32, tag="ofin")
            nc.scalar.activation(ofin[:], ps_1[:], RELU)
            nc.sync.dma_start(out_d[:], ofin[:])

    nc.compile()
    return nc


def make_in_maps(inputs: dict) -> list[dict]:
    feat16 = np.asarray(inputs["features"]).astype(np.float16)
    s0 = np.asarray(inputs["samples0"]).astype(np.int64).reshape(B)
    s1 = np.asarray(inputs["samples1"]).astype(np.int64).reshape(B, S1)
    s2 = np.asarray(inputs["samples2"]).astype(np.int64).reshape(B, S1, S2)
    ws0 = np.asarray(inputs["w_self0"], dtype=np.float32)
    wn0 = np.asarray(inputs["w_neigh0"], dtype=np.float32)
    ws1 = np.asarray(inputs["w_self1"], dtype=np.float32)
    wn1 = np.asarray(inputs["w_neigh1"], dtype=np.float32)

    # order must match W_NAMES; scales folded: wn0s2 = wn0/S2 (n1 neigh),
    # wn0s1 = wn0/S1 (n0 neigh, h1 tree is a plain sum), wn1*/S1.
    w_cat = np.concatenate([
        ws0, wn0 / S2, wn0 / S1, ws1[:D], ws1[D:], wn1[:D] / S1,
        wn1[D:] / S1, np.eye(D, dtype=np.float32),
    ], axis=1).astype(np.float16)   # [128, 8*128]

    in_maps = []
    for c in range(N_CORES):
        b0 = c * BL
        # h2T: per chunk, cols = (s2, s1_local, batch)
        s2c = s2[b0:b0 + BL]                         # [BL, S1, S2]
        idx_parts = []
        for cc in range(H2_CHUNKS):
            o, nsl = CHUNK_OFF[cc], CHUNK_SLS[cc]
            blk = s2c[:, o:o + nsl, :]               # [BL, nsl, S2]
            idx_parts.append(blk.transpose(2, 1, 0).reshape(-1))
        ids2 = np.concatenate(idx_parts)
        h2T = np.ascontiguousarray(feat16[ids2].T)   # [128, 32000]
        ids1 = s1[b0:b0 + BL].T.reshape(-1)          # (s1, b) flat
        h1T = np.ascontiguousarray(feat16[ids1].T)   # [128, 3200]
        h0T = feat16[s0[b0:b0 + BL]].T               # [128, 128]
        h0w = np.ascontiguousarray(
            np.concatenate([h0T, w_cat], axis=1))    # [128, 1152]
        in_maps.append(dict(h2=h2T, h1=h1T, h0w=h0w))
    return in_maps


_NC_CACHE = None


def _get_nc() -> bass.Bass:
    global _NC_CACHE
    if _NC_CACHE is None:
        _NC_CACHE = build_bass()
    return _NC_CACHE


def run(inputs: dict, trace: bool = False):
    """Returns (full_output [1024, 256] f32, BassKernelResults)."""
    in_maps = make_in_maps(inputs)
    res = run_bass_kernel_spmd(
        _get_nc(), in_maps, core_ids=list(range(N_CORES)), trace=trace
    )
    # device out r[j, half*128+b] -> out[b, half*128+j]
    outs = []
    for r in res.results:
        r2 = np.asarray(r["out"]).reshape(D, 2, BL)
        outs.append(r2.transpose(2, 1, 0).reshape(BL, 2 * D))
    return np.concatenate(outs, axis=0), res


def kernel(**inputs) -> np.ndarray:
    out, _ = run(inputs)
    return out
